# revision 80
# baseline (speedup 1.0000x reference)
"""Trainium2 Bass kernel for nn_CBAE_EndToEnd — 2D-tile active-prim
compaction, segmented-product design (v2).

Each 8x16-pixel tile (128 pixels on partitions) only intersects ~7 of
the 128 primitives (exact per-row x-interval test with sigmoid
saturation margin, OR'd over the tile's 8 rows and intersected with the
tile's x-range).  Host packs, per (frame, tile), the active prims into
a pitch-13 fp16 matmul stream per prim:
  [x-col: logit(aeff) | e0..e11 edge cols]
with contract-6 lhsT [ox, ox, oy, oy, 1, 1] (within-tile pixel offsets,
exact in fp16) and per-column coefficient rows [Ah, Al, Bh, Bl, Cth,
Ctl]; the tile corner is folded into Ct so the lhsT is static across
all tiles/frames.  sigma(x-col) = aeff folds opacity into the product.

Device per frame (tiles sorted by active count, uniform capacity C per
group; one contiguous stream DMA + matmul/sigmoid sweep per frame):
  PE    : arg = A*ox + B*oy + Ct via contract-6 fp16 matmul, static lhsT.
  ACT   : sigmoid over 3-PSUM-bank batches.
  DVE   : segmented product a = prod_13 sigma via tensor_reduce(mult)
          (1 elem/cycle vs 2 for the old scan); per-group compositing
          scan over (C+1)-pitch om.  C==1 groups skip compositing (w=a).
  Pool  : stream DMA issue, om = 1-a (strided), w = t0-t1 subtract.
  PE    : tight fp16 transposes of w (floor(128/C) slots per chunk), one
          3*ns-col color matmul per chunk into a shared PSUM bank.
  ACT/DVE: PSUM->SBUF copies (alternating); framebuffers accumulate in
          SBUF, ONE contiguous output DMA at the end.
Output [pix, frame*slot*3] pixel-major; host un-permutes tiles.
"""

import numpy as np

H = 128
W = 128
N = 128
K = 12
SOFT = 0.01
T_TOTAL = 192
N_CORES = 8
F = T_TOTAL // N_CORES
MARGIN = 4.75          # |arg| beyond this counts as saturated
PITCH = K + 1          # x-col + 12 edges (no reset col needed)
TY, TX = 8, 16         # tile shape in pixels
NTY, NTX = H // TY, W // TX
NSLOT = NTY * NTX      # 128 tiles, one slot each
BANK = 512             # fp32 cols per PSUM bank
GLIM = 4096            # max matmul-stream cols per group (8 banks)

fp16 = np.float16

_CACHE = {}


# ---------------------------------------------------------------------------
# host prep
# ---------------------------------------------------------------------------

def _split2(x):
    x = np.asarray(x, np.float32)
    h = x.astype(fp16)
    l = (x - h.astype(np.float32)).astype(fp16)
    return h, l


def _make_groups(capr, ovh=400):
    """DP-optimal grouping of sorted slots: uniform cap per group.
    capr[r] = max over frames of r-th smallest count.  Minimizes
    bank-rounded stream columns + a per-group fixed overhead."""
    NS = len(capr)
    INF = float("inf")
    cost = [0.0] + [INF] * NS
    prev = [0] * (NS + 1)
    for i in range(1, NS + 1):
        C = max(1, int(capr[i - 1]))
        for j in range(i - 1, -1, -1):
            cols = PITCH * C * (i - j)
            if cols > GLIM:
                break
            nb = (cols + BANK - 1) // BANK
            c = cost[j] + nb * BANK + ovh
            if c < cost[i]:
                cost[i] = c
                prev[i] = j
    bounds = []
    i = NS
    while i > 0:
        j = prev[i]
        bounds.append((j, i - j, max(1, int(capr[i - 1]))))
        i = j
    bounds.reverse()
    groups = []
    col_off = 0
    ck_off = 0
    for s0, n, C in bounds:
        cols = PITCH * C * n
        nb = (cols + BANK - 1) // BANK
        groups.append(dict(s0=s0, n=n, C=C, off=col_off, nb=nb,
                           spc=max(1, 128 // C), ckoff=ck_off))
        col_off += nb * BANK
        ck_off += 3 * n
    return groups, col_off // BANK, ck_off


def _groups_key(groups):
    return tuple((g["s0"], g["n"], g["C"]) for g in groups)


def _plan(trajectory, alpha, z, csg, colors):
    """Compute compaction plan + packed per-frame data for ALL frames."""
    T = trajectory.shape[0]
    od = np.argsort(z, kind="stable")[::-1]     # descending z = paint order
    traj = np.asarray(trajectory, np.float32)[:, 0, :]
    P = traj[:, : N * K * 2].reshape(T, N, K, 2)[:, od]
    alive = traj[:, N * K * 2:][:, od]
    v0 = P
    v1 = np.roll(P, -1, axis=2)
    e = v1 - v0
    area2 = np.sum(v0[..., 0] * v1[..., 1] - v1[..., 0] * v0[..., 1], axis=2)
    orient = np.sign(area2).astype(np.float32)[:, :, None]
    A = (-orient * e[..., 1] / SOFT).astype(np.float32)       # [T,N,K] gx coef
    B = (orient * e[..., 0] / SOFT).astype(np.float32)        # gy coef
    Cc = (orient * (e[..., 1] * v0[..., 0] - e[..., 0] * v0[..., 1]) / SOFT
          ).astype(np.float32)

    sig_alive = 1.0 / (1.0 + np.exp(-alive.astype(np.float32)))
    aeff = np.asarray(alpha, np.float32)[od][None, :] * sig_alive   # [T,N]
    aeff = np.clip(aeff, 1e-12, 1.0 - 1e-7)
    logit = np.log(aeff / (1.0 - aeff)).astype(np.float32)          # [T,N]
    ck = (np.asarray(colors, np.float32)[0][od]
          * (1.0 - np.asarray(csg)[od].astype(np.float32))[:, None])  # [N,3]

    ys = ((np.arange(H) + 0.5) / H).astype(np.float32)
    x0, x1 = 0.5 / W, (W - 0.5) / W

    # --- per-row exact feasible x-interval, then per-tile activity:
    # a prim is active in a tile iff some row of the tile has a feasible
    # x-interval intersecting the tile's x-range.
    cx0 = ((np.arange(NTX) * TX + 0.5) / W).astype(np.float32)
    cx1 = ((np.arange(NTX) * TX + TX - 0.5) / W).astype(np.float32)
    cnt = np.empty((T, NSLOT), np.int32)
    active = np.empty((T, N, NSLOT), bool)
    step = 32
    for t0 in range(0, T, step):
        sl = slice(t0, t0 + step)
        D = B[sl, :, :, None] * ys[None, None, None, :] + Cc[sl, :, :, None]
        Ae = A[sl, :, :, None]
        Asafe = np.where(Ae == 0, 1.0, Ae)
        lo = np.where(Ae > 0, (-MARGIN - D) / Asafe, x0)
        hi = np.where(Ae < 0, (-MARGIN - D) / Asafe, x1)
        lo = np.where((Ae == 0) & (D < -MARGIN), x1 + 1.0, lo)
        LO = np.maximum(x0, lo.max(axis=2))       # [t,N,H]
        HI = np.minimum(x1, hi.min(axis=2))
        LOr = LO.reshape(-1, N, NTY, TY)
        HIr = HI.reshape(-1, N, NTY, TY)
        act = (np.maximum(LOr[..., None], cx0[None, None, None, None, :])
               <= np.minimum(HIr[..., None], cx1[None, None, None, None, :])
               ).any(axis=3)                      # [t,N,NTY,NTX]
        act = act.reshape(-1, N, NSLOT)
        active[sl] = act
        cnt[sl] = act.sum(axis=1)

    # --- slots: tiles sorted ascending by count; adaptive groups
    order = np.argsort(cnt, axis=1, kind="stable")       # [T, NSLOT]
    scnt = np.take_along_axis(cnt, order, axis=1)
    capr = scnt.max(axis=0)                              # [NSLOT]
    groups, NBTOT, CKTOT = _make_groups(capr)

    # active prim indices per (t, tile), z-order preserved
    Cmax = max(g["C"] for g in groups)
    act_tr = np.transpose(active, (0, 2, 1))             # [T, S, N]
    idx = np.argsort(~act_tr, axis=2, kind="stable")[:, :, :Cmax]  # [T,S,Cmax]
    valid = np.take_along_axis(act_tr, idx, axis=2)      # [T,S,Cmax]

    Ah, Al = _split2(A)
    Bh, Bl = _split2(B)
    w6 = np.zeros((T, 6, NBTOT * BANK), fp16)
    ck2a = np.zeros((T, 128, max(CKTOT, 1)), fp16)
    tt = np.arange(T)[:, None, None]
    for g in groups:
        C, off, n_g, spc, ckoff = (g["C"], g["off"], g["n"],
                                   g["spc"], g["ckoff"])
        slots = order[:, g["s0"]:g["s0"] + n_g]           # [T, n_g] tile ids
        pidx = np.take_along_axis(idx, slots[:, :, None], axis=1)[:, :, :C]
        pval = np.take_along_axis(valid, slots[:, :, None], axis=1)[:, :, :C]
        Ah_g = Ah[tt, pidx].astype(np.float32)            # [T,n,C,K]
        Al_g = Al[tt, pidx].astype(np.float32)
        Bh_g = Bh[tt, pidx].astype(np.float32)
        Bl_g = Bl[tt, pidx].astype(np.float32)
        A_g = A[tt, pidx]
        B_g = B[tt, pidx]
        Cc_g = Cc[tt, pidx]
        tx0s = ((slots % NTX) * TX / W).astype(np.float32)[:, :, None, None]
        ty0s = ((slots // NTX) * TY / H).astype(np.float32)[:, :, None, None]
        Ct = (A_g * tx0s + B_g * ty0s + Cc_g).astype(np.float32)
        Cth, Ctl = _split2(Ct)
        lg = logit[tt, pidx].astype(np.float32)           # [T,n,C]
        lg = np.where(pval, lg, -30.0)
        lgh, lgl = _split2(lg)
        dead = ~pval[..., None]
        Ah_g = np.where(dead, 0.0, Ah_g)
        Al_g = np.where(dead, 0.0, Al_g)
        Bh_g = np.where(dead, 0.0, Bh_g)
        Bl_g = np.where(dead, 0.0, Bl_g)
        Cth = np.where(dead, fp16(0), Cth)
        Ctl = np.where(dead, fp16(0), Ctl)

        blk = np.zeros((T, n_g, C, PITCH, 6), fp16)
        blk[..., 0, 4] = lgh                    # x col: Ct = logit
        blk[..., 0, 5] = lgl
        blk[..., 1:1 + K, 0] = Ah_g
        blk[..., 1:1 + K, 1] = Al_g
        blk[..., 1:1 + K, 2] = Bh_g
        blk[..., 1:1 + K, 3] = Bl_g
        blk[..., 1:1 + K, 4] = Cth
        blk[..., 1:1 + K, 5] = Ctl
        flat = blk.reshape(T, n_g * C * PITCH, 6)
        w6[:, :, off:off + flat.shape[1]] = flat.transpose(0, 2, 1)

        ckg = ck[pidx] * pval[..., None]                  # [T,n,C,3]
        for s in range(n_g):
            part0 = (s % spc) * C
            ck2a[:, part0:part0 + C, ckoff + 3 * s:ckoff + 3 * s + 3] = \
                ckg[:, s]

    # static lhsT: within-tile pixel offsets (exact in fp16)
    p = np.arange(128)
    ox = ((p % TX + 0.5) / W).astype(fp16)
    oy = ((p // TX + 0.5) / H).astype(fp16)
    G6 = np.zeros((6, 128), fp16)
    G6[0] = ox
    G6[1] = ox
    G6[2] = oy
    G6[3] = oy
    G6[4] = 1.0
    G6[5] = 1.0
    ident = np.eye(128, dtype=fp16)

    return dict(w6=w6, ck2a=ck2a, G6=G6, ident=ident, groups=groups,
                nbtot=NBTOT, cktot=max(CKTOT, 1), order=order)


def _host_prep(trajectory, colors, alpha, z, csg):
    plan = _plan(trajectory, alpha, z, csg, colors)
    in_maps = []
    for c in range(N_CORES):
        fr = slice(c * F, (c + 1) * F)
        ck = plan["ck2a"][fr]                      # [F, 128, cktot]
        ck = ck.transpose(1, 0, 2).reshape(128, -1)  # [128, F*cktot]
        in_maps.append({
            "g6": np.ascontiguousarray(plan["G6"]),
            "ident": np.ascontiguousarray(plan["ident"]),
            "w6": np.ascontiguousarray(plan["w6"][fr]),
            "ck2a": np.ascontiguousarray(ck),
        })
    return in_maps, plan


# ---------------------------------------------------------------------------
# device program
# ---------------------------------------------------------------------------

def _build_nc(n_frames, groups, nbtot, cktot):
    import concourse.bass as bass
    import concourse.bacc as bacc
    import concourse.tile as tile
    from concourse import mybir
    from contextlib import ExitStack

    dt = mybir.dt
    AF = mybir.ActivationFunctionType
    ALU = mybir.AluOpType
    AX = mybir.AxisListType

    nc = bacc.Bacc(None)
    g6_d = nc.dram_tensor("g6", [6, 128], dt.float16, kind="ExternalInput")
    ident_d = nc.dram_tensor("ident", [128, 128], dt.float16,
                             kind="ExternalInput")
    w6_d = nc.dram_tensor("w6", [n_frames, 6, nbtot * BANK], dt.float16,
                          kind="ExternalInput")
    ck2a_d = nc.dram_tensor("ck2a", [128, n_frames * cktot], dt.float16,
                            kind="ExternalInput")
    out_d = nc.dram_tensor("out", [128, n_frames * NSLOT * 3], dt.float32,
                           kind="ExternalOutput")

    with tile.TileContext(nc) as tc:
        with ExitStack() as ctx:
            singles = ctx.enter_context(tc.tile_pool(name="singles", bufs=1))
            w6_pool = ctx.enter_context(tc.tile_pool(name="w6", bufs=3))
            sp_pool = ctx.enter_context(tc.tile_pool(name="sp", bufs=3))
            a_pool = ctx.enter_context(tc.tile_pool(name="a", bufs=3))
            om_pool = ctx.enter_context(tc.tile_pool(name="om", bufs=3))
            tt_pool = ctx.enter_context(tc.tile_pool(name="tt", bufs=3))
            w_pool = ctx.enter_context(tc.tile_pool(name="w", bufs=3))
            wt_pool = ctx.enter_context(tc.tile_pool(name="wt", bufs=3))
            s_psum = ctx.enter_context(
                tc.tile_pool(name="s_ps", bufs=2, space="PSUM"))
            t_psum = ctx.enter_context(
                tc.tile_pool(name="t_ps", bufs=1, space="PSUM"))
            c_psum = ctx.enter_context(
                tc.tile_pool(name="c_ps", bufs=1, space="PSUM"))

            g6_sb = singles.tile([6, 128], dt.float16)
            nc.sync.dma_start(out=g6_sb, in_=g6_d[:])
            ones16 = singles.tile([128, 1], dt.float16)
            nc.vector.memset(ones16, 1.0)
            ident_sb = singles.tile([128, 128], dt.float16)
            nc.sync.dma_start(out=ident_sb, in_=ident_d[:])
            # preload all frames' colors once (per-frame [128, cktot] DMAs
            # cost ~21us of descriptor generation each on the sync queue)
            ck_all = singles.tile([128, n_frames * cktot], dt.float16)
            nc.sync.dma_start(out=ck_all, in_=ck2a_d[:])
            # all frames' framebuffers accumulate here; ONE contiguous DMA
            # at the end (per-frame [slot,pix,ch]-scatter DMAs cost ~30us
            # each in 12-byte runs)
            fb_all = singles.tile([128, n_frames * NSLOT * 3], dt.float32)
            d1b = {}
            for gi, g in enumerate(groups):
                C, n_g = g["C"], g["n"]
                t2 = singles.tile([128, n_g * (C + 1)], dt.float16,
                                  tag=f"d1b{gi}")
                nc.vector.memset(t2, 0.0)
                r2 = bass.AP(tensor=t2.tensor, offset=t2.offset,
                             ap=[t2.ap[0], [C + 1, n_g], [1, 1]])
                nc.vector.memset(r2, 1.0)
                d1b[gi] = t2

            prev_fb = None
            for t in range(n_frames):
                cko = t * cktot
                co_ps = c_psum.tile([128, BANK], dt.float32, tag="co")
                # front half: stream matmuls + sigmoid + segmented product.
                # Emitted for ALL groups before any transposes so the PE
                # queue never stalls on a transpose waiting for the DVE
                # chain of an earlier group.  Compositing (om/scan/w) for
                # group gi-1 is interleaved after front(gi) so the DVE
                # queue reaches each scan only after its om is ready, and
                # the scans complete early enough for the transposes.
                a_sbs = {}
                w_sbs = {}
                # one contiguous stream DMA + matmul/sigmoid sweep for the
                # whole frame (groups are adjacent in DRAM); per-group work
                # starts at the segmented product
                gcols_all = nbtot * BANK
                w6_sb = w6_pool.tile([6, gcols_all], dt.float16, tag="w6")
                with tc.high_priority():
                    nc.gpsimd.dma_start(out=w6_sb, in_=w6_d[t])
                sp_sb = sp_pool.tile([128, gcols_all], dt.float16, tag="sp")
                lastg = groups[-1]
                tot_used = lastg["off"] + PITCH * lastg["C"] * lastg["n"]
                for b0 in range(0, nbtot, 3):
                    nbk = min(3, nbtot - b0)
                    s_ps = s_psum.tile([128, 3 * BANK], dt.float32, tag="s")
                    for b in range(nbk):
                        nc.tensor.matmul(
                            s_ps[:, b * BANK:(b + 1) * BANK],
                            lhsT=g6_sb,
                            rhs=w6_sb[:, (b0 + b) * BANK:(b0 + b + 1) * BANK],
                            start=True, stop=True)
                    ncols = min(nbk * BANK, tot_used - b0 * BANK)
                    nc.scalar.activation(
                        sp_sb[:, b0 * BANK:b0 * BANK + ncols],
                        s_ps[:, :ncols], AF.Sigmoid)

                def composite(gi):
                    g = groups[gi]
                    C, n_g = g["C"], g["n"]
                    a_sb = a_sbs[gi]
                    if C == 1:
                        # single prim per tile: w = a, no compositing
                        w_sbs[gi] = a_sb
                        return
                    om_sb = om_pool.tile([128, n_g * (C + 1)], dt.float16,
                                         tag="om")
                    r2 = bass.AP(tensor=om_sb.tensor, offset=om_sb.offset,
                                 ap=[om_sb.ap[0], [C + 1, n_g], [1, 1]])
                    nc.gpsimd.memset(r2, 1.0)
                    om_ap = bass.AP(tensor=om_sb.tensor,
                                    offset=om_sb.offset + 1,
                                    ap=[om_sb.ap[0], [C + 1, n_g], [1, C]])
                    nc.gpsimd.tensor_scalar(om_ap, a_sb, -1.0, 1.0,
                                            ALU.mult, ALU.add)
                    tt_sb = tt_pool.tile([128, n_g * (C + 1)], dt.float16,
                                         tag="tt")
                    nc.vector.tensor_tensor_scan(
                        out=tt_sb, data0=om_sb, data1=d1b[gi],
                        initial=ones16[:, 0:1], op0=ALU.mult, op1=ALU.max)
                    w_sb = w_pool.tile([128, n_g * C], dt.float16, tag="w")
                    t0_ap = bass.AP(tensor=tt_sb.tensor, offset=tt_sb.offset,
                                    ap=[tt_sb.ap[0], [C + 1, n_g], [1, C]])
                    t1_ap = bass.AP(tensor=tt_sb.tensor,
                                    offset=tt_sb.offset + 1,
                                    ap=[tt_sb.ap[0], [C + 1, n_g], [1, C]])
                    nc.gpsimd.tensor_tensor(w_sb, t0_ap, t1_ap, ALU.subtract)
                    w_sbs[gi] = w_sb

                gorder = list(range(len(groups)))
                for gi in gorder:
                    g = groups[gi]
                    C, off, n_g = g["C"], g["off"], g["n"]
                    # segmented product over pitch-13 segments
                    a_sb = a_pool.tile([128, C * n_g], dt.float16,
                                       tag=f"a{gi}")
                    sp_ap = bass.AP(tensor=sp_sb.tensor,
                                    offset=sp_sb.offset + off,
                                    ap=[sp_sb.ap[0], [PITCH, C * n_g],
                                        [1, PITCH]])
                    nc.vector.tensor_reduce(a_sb, sp_ap, AX.X, ALU.mult)
                    a_sbs[gi] = a_sb
                # previous frame's framebuffer drain: its color matmuls
                # complete before this frame's stream matmuls do (PE FIFO),
                # so placed here it never blocks this frame's sigmoids
                if prev_fb is not None:
                    tp, co = prev_fb
                    nc.scalar.copy(
                        fb_all[:, tp * NSLOT * 3:(tp + 1) * NSLOT * 3],
                        co[:, :NSLOT * 3])
                for gi in gorder:
                    composite(gi)
                # transposes + color matmuls
                chunk_i = 0
                for gi in gorder:
                    g = groups[gi]
                    C, n_g = g["C"], g["n"]
                    spc, ckoff = g["spc"], g["ckoff"]
                    w_sb = w_sbs[gi]
                    nchunk = (n_g + spc - 1) // spc
                    for j in range(nchunk):
                        ns = min(spc, n_g - j * spc)
                        ccols = ns * C
                        wt_ps = t_psum.tile([128, 1024], dt.float16, tag="wt")
                        nc.tensor.transpose(
                            wt_ps[:ccols, :128],
                            w_sb[:, j * spc * C:j * spc * C + ccols],
                            ident_sb)
                        wt_sb = wt_pool.tile([128, 128], dt.float16, tag="wts")
                        if chunk_i % 3 == 0:
                            nc.scalar.copy(wt_sb[:ccols, :],
                                           wt_ps[:ccols, :128])
                        else:
                            nc.vector.tensor_copy(wt_sb[:ccols, :],
                                                  wt_ps[:ccols, :128])
                        chunk_i += 1
                        s0 = j * spc
                        nc.tensor.matmul(
                            co_ps[:, (g["s0"] + s0) * 3:
                                  (g["s0"] + s0 + ns) * 3],
                            lhsT=wt_sb[0:ccols, :],
                            rhs=ck_all[0:ccols,
                                       cko + ckoff + 3 * s0:
                                       cko + ckoff + 3 * (s0 + ns)],
                            start=True, stop=True)
                prev_fb = (t, co_ps)
            tp, co = prev_fb
            nc.scalar.copy(
                fb_all[:, tp * NSLOT * 3:(tp + 1) * NSLOT * 3],
                co[:, :NSLOT * 3])
            nc.sync.dma_start(out=out_d[:], in_=fb_all)
    nc.finalize()
    return nc


def _get_program(n_frames, groups, nbtot, cktot):
    key = (n_frames, _groups_key(groups), nbtot, cktot)
    if key not in _CACHE:
        _CACHE[key] = _build_nc(n_frames, groups, nbtot, cktot)
    return _CACHE[key]


def _enable_jax_cache():
    try:
        import jax
        if jax.config.jax_compilation_cache_dir is None:
            jax.config.update("jax_compilation_cache_dir", "/tmp/jax_bass_cache")
            jax.config.update("jax_persistent_cache_min_entry_size_bytes", -1)
            jax.config.update("jax_persistent_cache_min_compile_time_secs", 0.5)
    except Exception:
        pass


def _assemble(outs, order):
    """outs: per-core [128, F*NSLOT*3] device outputs -> [T, slot, pix, 3]."""
    devs = []
    for o in outs:
        d = o.reshape(128, -1, NSLOT, 3)          # [pix, F, slot, ch]
        devs.append(np.ascontiguousarray(d.transpose(1, 2, 0, 3)))
    return np.concatenate(devs, axis=0)


def _unpermute(dev, order):
    """dev [T, slot, 128, 3] -> video [T, H, W, 3] (tile un-permute)."""
    T = dev.shape[0]
    video = np.empty((T, NSLOT, TY, TX, 3), np.float32)
    tt = np.arange(T)[:, None]
    video[tt, order] = dev.reshape(T, NSLOT, TY, TX, 3)
    video = video.reshape(T, NTY, NTX, TY, TX, 3)
    video = video.transpose(0, 1, 3, 2, 4, 5).reshape(T, H, W, 3)
    return video


def kernel(trajectory, colors, alpha, z, csg):
    import time
    from concourse.bass_utils import run_bass_kernel_spmd

    _enable_jax_cache()

    in_maps, plan = _host_prep(
        np.asarray(trajectory), np.asarray(colors), np.asarray(alpha),
        np.asarray(z), np.asarray(csg))
    nc = _get_program(F, plan["groups"], plan["nbtot"], plan["cktot"])
    res = None
    for attempt in range(3):
        try:
            res = run_bass_kernel_spmd(nc, in_maps,
                                       core_ids=list(range(N_CORES)))
            break
        except Exception:
            # transient NRT_EXEC_UNIT_UNRECOVERABLE on first exec in a
            # fresh process; a retry has always succeeded
            if attempt == 2:
                raise
            time.sleep(2.0)
    outs = [res.results[c]["out"] for c in range(N_CORES)]
    dev = _assemble(outs, plan["order"])        # [192, slot, pix, 3]
    video = _unpermute(dev, plan["order"])
    return video[None].astype(np.float32)


if __name__ == "__main__":
    import time
    d = np.load("/root/problem/ref_cache.npz")
    t0 = time.time()
    in_maps, plan = _host_prep(d["trajectory"], d["colors"], d["alpha"],
                               d["z"], d["csg"])
    print(f"host prep: {time.time()-t0:.1f}s nbtot={plan['nbtot']}")
    print("groups:", _groups_key(plan["groups"]))
    t0 = time.time()
    nc = _build_nc(2, plan["groups"], plan["nbtot"], plan["cktot"])
    print(f"build 2f: {time.time()-t0:.1f}s")


# revision 81
# speedup vs baseline: 1.0399x; 1.0399x over previous
"""Trainium2 Bass kernel for nn_CBAE_EndToEnd — 2D-tile active-prim
compaction, segmented-product design (v2).

Each 8x16-pixel tile (128 pixels on partitions) only intersects ~7 of
the 128 primitives (exact per-row x-interval test with sigmoid
saturation margin, OR'd over the tile's 8 rows and intersected with the
tile's x-range).  Host packs, per (frame, tile), the active prims into
a pitch-13 fp16 matmul stream per prim:
  [x-col: logit(aeff) | e0..e11 edge cols]
with contract-6 lhsT [ox, ox, oy, oy, 1, 1] (within-tile pixel offsets,
exact in fp16) and per-column coefficient rows [Ah, Al, Bh, Bl, Cth,
Ctl]; the tile corner is folded into Ct so the lhsT is static across
all tiles/frames.  sigma(x-col) = aeff folds opacity into the product.

Device per frame (tiles sorted by active count, uniform capacity C per
group; one contiguous stream DMA + matmul/sigmoid sweep per frame):
  PE    : arg = A*ox + B*oy + Ct via contract-6 fp16 matmul, static lhsT.
  ACT   : sigmoid over 3-PSUM-bank batches.
  DVE   : segmented product a = prod_13 sigma via tensor_reduce(mult)
          (1 elem/cycle vs 2 for the old scan); per-group compositing
          scan over (C+1)-pitch om.  C==1 groups skip compositing (w=a).
  Pool  : stream DMA issue, om = 1-a (strided), w = t0-t1 subtract.
  PE    : tight fp16 transposes of w (floor(128/C) slots per chunk), one
          3*ns-col color matmul per chunk into a shared PSUM bank.
  ACT/DVE: PSUM->SBUF copies (alternating); framebuffers accumulate in
          SBUF, ONE contiguous output DMA at the end.
Output [pix, frame*slot*3] pixel-major; host un-permutes tiles.
"""

import numpy as np

H = 128
W = 128
N = 128
K = 12
SOFT = 0.01
T_TOTAL = 192
N_CORES = 8
F = T_TOTAL // N_CORES
MARGIN = 4.75          # |arg| beyond this counts as saturated
PITCH = K + 1          # x-col + 12 edges (no reset col needed)
TY, TX = 8, 16         # tile shape in pixels
NTY, NTX = H // TY, W // TX
NSLOT = NTY * NTX      # 128 tiles, one slot each
BANK = 512             # fp32 cols per PSUM bank
GLIM = 4096            # max matmul-stream cols per group (8 banks)

fp16 = np.float16

_CACHE = {}


# ---------------------------------------------------------------------------
# host prep
# ---------------------------------------------------------------------------

def _split2(x):
    x = np.asarray(x, np.float32)
    h = x.astype(fp16)
    l = (x - h.astype(np.float32)).astype(fp16)
    return h, l


def _make_groups(capr, ovh=400):
    """DP-optimal grouping of sorted slots: uniform cap per group.
    capr[r] = max over frames of r-th smallest count.  Minimizes
    bank-rounded stream columns + a per-group fixed overhead."""
    NS = len(capr)
    INF = float("inf")
    cost = [0.0] + [INF] * NS
    prev = [0] * (NS + 1)
    for i in range(1, NS + 1):
        C = max(1, int(capr[i - 1]))
        for j in range(i - 1, -1, -1):
            cols = PITCH * C * (i - j)
            if cols > GLIM:
                break
            nb = (cols + BANK - 1) // BANK
            c = cost[j] + nb * BANK + ovh
            if c < cost[i]:
                cost[i] = c
                prev[i] = j
    bounds = []
    i = NS
    while i > 0:
        j = prev[i]
        bounds.append((j, i - j, max(1, int(capr[i - 1]))))
        i = j
    bounds.reverse()
    groups = []
    col_off = 0
    ck_off = 0
    for s0, n, C in bounds:
        cols = PITCH * C * n
        nb = (cols + BANK - 1) // BANK
        groups.append(dict(s0=s0, n=n, C=C, off=col_off, nb=nb,
                           spc=max(1, 128 // C), ckoff=ck_off))
        col_off += nb * BANK
        ck_off += 3 * n
    return groups, col_off // BANK, ck_off


def _groups_key(groups):
    return tuple((g["s0"], g["n"], g["C"]) for g in groups)


def _plan(trajectory, alpha, z, csg, colors):
    """Compute compaction plan + packed per-frame data for ALL frames."""
    T = trajectory.shape[0]
    od = np.argsort(z, kind="stable")[::-1]     # descending z = paint order
    traj = np.asarray(trajectory, np.float32)[:, 0, :]
    P = traj[:, : N * K * 2].reshape(T, N, K, 2)[:, od]
    alive = traj[:, N * K * 2:][:, od]
    v0 = P
    v1 = np.roll(P, -1, axis=2)
    e = v1 - v0
    area2 = np.sum(v0[..., 0] * v1[..., 1] - v1[..., 0] * v0[..., 1], axis=2)
    orient = np.sign(area2).astype(np.float32)[:, :, None]
    A = (-orient * e[..., 1] / SOFT).astype(np.float32)       # [T,N,K] gx coef
    B = (orient * e[..., 0] / SOFT).astype(np.float32)        # gy coef
    Cc = (orient * (e[..., 1] * v0[..., 0] - e[..., 0] * v0[..., 1]) / SOFT
          ).astype(np.float32)

    sig_alive = 1.0 / (1.0 + np.exp(-alive.astype(np.float32)))
    aeff = np.asarray(alpha, np.float32)[od][None, :] * sig_alive   # [T,N]
    aeff = np.clip(aeff, 1e-12, 1.0 - 1e-7)
    logit = np.log(aeff / (1.0 - aeff)).astype(np.float32)          # [T,N]
    ck = (np.asarray(colors, np.float32)[0][od]
          * (1.0 - np.asarray(csg)[od].astype(np.float32))[:, None])  # [N,3]

    ys = ((np.arange(H) + 0.5) / H).astype(np.float32)
    x0, x1 = 0.5 / W, (W - 0.5) / W

    # --- per-row exact feasible x-interval, then per-tile activity:
    # a prim is active in a tile iff some row of the tile has a feasible
    # x-interval intersecting the tile's x-range.
    cx0 = ((np.arange(NTX) * TX + 0.5) / W).astype(np.float32)
    cx1 = ((np.arange(NTX) * TX + TX - 0.5) / W).astype(np.float32)
    cnt = np.empty((T, NSLOT), np.int32)
    active = np.empty((T, N, NSLOT), bool)
    step = 32
    for t0 in range(0, T, step):
        sl = slice(t0, t0 + step)
        D = B[sl, :, :, None] * ys[None, None, None, :] + Cc[sl, :, :, None]
        Ae = A[sl, :, :, None]
        Asafe = np.where(Ae == 0, 1.0, Ae)
        lo = np.where(Ae > 0, (-MARGIN - D) / Asafe, x0)
        hi = np.where(Ae < 0, (-MARGIN - D) / Asafe, x1)
        lo = np.where((Ae == 0) & (D < -MARGIN), x1 + 1.0, lo)
        LO = np.maximum(x0, lo.max(axis=2))       # [t,N,H]
        HI = np.minimum(x1, hi.min(axis=2))
        LOr = LO.reshape(-1, N, NTY, TY)
        HIr = HI.reshape(-1, N, NTY, TY)
        act = (np.maximum(LOr[..., None], cx0[None, None, None, None, :])
               <= np.minimum(HIr[..., None], cx1[None, None, None, None, :])
               ).any(axis=3)                      # [t,N,NTY,NTX]
        act = act.reshape(-1, N, NSLOT)
        active[sl] = act
        cnt[sl] = act.sum(axis=1)

    # --- slots: tiles sorted ascending by count; adaptive groups
    order = np.argsort(cnt, axis=1, kind="stable")       # [T, NSLOT]
    scnt = np.take_along_axis(cnt, order, axis=1)
    capr = scnt.max(axis=0)                              # [NSLOT]
    groups, NBTOT, CKTOT = _make_groups(capr)

    # active prim indices per (t, tile), z-order preserved
    Cmax = max(g["C"] for g in groups)
    act_tr = np.transpose(active, (0, 2, 1))             # [T, S, N]
    idx = np.argsort(~act_tr, axis=2, kind="stable")[:, :, :Cmax]  # [T,S,Cmax]
    valid = np.take_along_axis(act_tr, idx, axis=2)      # [T,S,Cmax]

    Ah, Al = _split2(A)
    Bh, Bl = _split2(B)
    w6 = np.zeros((T, 6, NBTOT * BANK), fp16)
    ck2a = np.zeros((T, 128, max(CKTOT, 1)), fp16)
    tt = np.arange(T)[:, None, None]
    for g in groups:
        C, off, n_g, spc, ckoff = (g["C"], g["off"], g["n"],
                                   g["spc"], g["ckoff"])
        slots = order[:, g["s0"]:g["s0"] + n_g]           # [T, n_g] tile ids
        pidx = np.take_along_axis(idx, slots[:, :, None], axis=1)[:, :, :C]
        pval = np.take_along_axis(valid, slots[:, :, None], axis=1)[:, :, :C]
        Ah_g = Ah[tt, pidx].astype(np.float32)            # [T,n,C,K]
        Al_g = Al[tt, pidx].astype(np.float32)
        Bh_g = Bh[tt, pidx].astype(np.float32)
        Bl_g = Bl[tt, pidx].astype(np.float32)
        A_g = A[tt, pidx]
        B_g = B[tt, pidx]
        Cc_g = Cc[tt, pidx]
        tx0s = ((slots % NTX) * TX / W).astype(np.float32)[:, :, None, None]
        ty0s = ((slots // NTX) * TY / H).astype(np.float32)[:, :, None, None]
        Ct = (A_g * tx0s + B_g * ty0s + Cc_g).astype(np.float32)
        Cth, Ctl = _split2(Ct)
        lg = logit[tt, pidx].astype(np.float32)           # [T,n,C]
        lg = np.where(pval, lg, -30.0)
        lgh, lgl = _split2(lg)
        dead = ~pval[..., None]
        Ah_g = np.where(dead, 0.0, Ah_g)
        Al_g = np.where(dead, 0.0, Al_g)
        Bh_g = np.where(dead, 0.0, Bh_g)
        Bl_g = np.where(dead, 0.0, Bl_g)
        Cth = np.where(dead, fp16(0), Cth)
        Ctl = np.where(dead, fp16(0), Ctl)

        blk = np.zeros((T, n_g, C, PITCH, 6), fp16)
        blk[..., 0, 4] = lgh                    # x col: Ct = logit
        blk[..., 0, 5] = lgl
        blk[..., 1:1 + K, 0] = Ah_g
        blk[..., 1:1 + K, 1] = Al_g
        blk[..., 1:1 + K, 2] = Bh_g
        blk[..., 1:1 + K, 3] = Bl_g
        blk[..., 1:1 + K, 4] = Cth
        blk[..., 1:1 + K, 5] = Ctl
        flat = blk.reshape(T, n_g * C * PITCH, 6)
        w6[:, :, off:off + flat.shape[1]] = flat.transpose(0, 2, 1)

        ckg = ck[pidx] * pval[..., None]                  # [T,n,C,3]
        for s in range(n_g):
            part0 = (s % spc) * C
            ck2a[:, part0:part0 + C, ckoff + 3 * s:ckoff + 3 * s + 3] = \
                ckg[:, s]

    # static lhsT: within-tile pixel offsets (exact in fp16)
    p = np.arange(128)
    ox = ((p % TX + 0.5) / W).astype(fp16)
    oy = ((p // TX + 0.5) / H).astype(fp16)
    G6 = np.zeros((6, 128), fp16)
    G6[0] = ox
    G6[1] = ox
    G6[2] = oy
    G6[3] = oy
    G6[4] = 1.0
    G6[5] = 1.0
    ident = np.eye(128, dtype=fp16)

    return dict(w6=w6, ck2a=ck2a, G6=G6, ident=ident, groups=groups,
                nbtot=NBTOT, cktot=max(CKTOT, 1), order=order)


def _host_prep(trajectory, colors, alpha, z, csg):
    plan = _plan(trajectory, alpha, z, csg, colors)
    in_maps = []
    for c in range(N_CORES):
        fr = slice(c * F, (c + 1) * F)
        ck = plan["ck2a"][fr]                      # [F, 128, cktot]
        ck = ck.transpose(1, 0, 2).reshape(128, -1)  # [128, F*cktot]
        in_maps.append({
            "g6": np.ascontiguousarray(plan["G6"]),
            "ident": np.ascontiguousarray(plan["ident"]),
            "w6": np.ascontiguousarray(plan["w6"][fr]),
            "ck2a": np.ascontiguousarray(ck),
        })
    return in_maps, plan


# ---------------------------------------------------------------------------
# device program
# ---------------------------------------------------------------------------

def _build_nc(n_frames, groups, nbtot, cktot):
    import concourse.bass as bass
    import concourse.bacc as bacc
    import concourse.tile as tile
    from concourse import mybir
    from contextlib import ExitStack

    dt = mybir.dt
    AF = mybir.ActivationFunctionType
    ALU = mybir.AluOpType
    AX = mybir.AxisListType

    nc = bacc.Bacc(None)
    g6_d = nc.dram_tensor("g6", [6, 128], dt.float16, kind="ExternalInput")
    ident_d = nc.dram_tensor("ident", [128, 128], dt.float16,
                             kind="ExternalInput")
    w6_d = nc.dram_tensor("w6", [n_frames, 6, nbtot * BANK], dt.float16,
                          kind="ExternalInput")
    ck2a_d = nc.dram_tensor("ck2a", [128, n_frames * cktot], dt.float16,
                            kind="ExternalInput")
    out_d = nc.dram_tensor("out", [128, n_frames * NSLOT * 3], dt.float32,
                           kind="ExternalOutput")

    with tile.TileContext(nc) as tc:
        with ExitStack() as ctx:
            singles = ctx.enter_context(tc.tile_pool(name="singles", bufs=1))
            w6_pool = ctx.enter_context(tc.tile_pool(name="w6", bufs=3))
            sp_pool = ctx.enter_context(tc.tile_pool(name="sp", bufs=3))
            a_pool = ctx.enter_context(tc.tile_pool(name="a", bufs=3))
            om_pool = ctx.enter_context(tc.tile_pool(name="om", bufs=3))
            tt_pool = ctx.enter_context(tc.tile_pool(name="tt", bufs=3))
            w_pool = ctx.enter_context(tc.tile_pool(name="w", bufs=3))
            wt_pool = ctx.enter_context(tc.tile_pool(name="wt", bufs=3))
            s_psum = ctx.enter_context(
                tc.tile_pool(name="s_ps", bufs=2, space="PSUM"))
            t_psum = ctx.enter_context(
                tc.tile_pool(name="t_ps", bufs=1, space="PSUM"))
            c_psum = ctx.enter_context(
                tc.tile_pool(name="c_ps", bufs=1, space="PSUM"))

            g6_sb = singles.tile([6, 128], dt.float16)
            nc.sync.dma_start(out=g6_sb, in_=g6_d[:])
            ones16 = singles.tile([128, 1], dt.float16)
            nc.vector.memset(ones16, 1.0)
            ident_sb = singles.tile([128, 128], dt.float16)
            nc.sync.dma_start(out=ident_sb, in_=ident_d[:])
            # preload all frames' colors once (per-frame [128, cktot] DMAs
            # cost ~21us of descriptor generation each on the sync queue)
            ck_all = singles.tile([128, n_frames * cktot], dt.float16)
            nc.sync.dma_start(out=ck_all, in_=ck2a_d[:])
            # all frames' framebuffers accumulate here; ONE contiguous DMA
            # at the end (per-frame [slot,pix,ch]-scatter DMAs cost ~30us
            # each in 12-byte runs)
            fb_all = singles.tile([128, n_frames * NSLOT * 3], dt.float32)
            d1b = {}
            for gi, g in enumerate(groups):
                C, n_g = g["C"], g["n"]
                t2 = singles.tile([128, n_g * (C + 1)], dt.float16,
                                  tag=f"d1b{gi}")
                nc.vector.memset(t2, 0.0)
                r2 = bass.AP(tensor=t2.tensor, offset=t2.offset,
                             ap=[t2.ap[0], [C + 1, n_g], [1, 1]])
                nc.vector.memset(r2, 1.0)
                d1b[gi] = t2

            prev_fb = None
            for t in range(n_frames):
                cko = t * cktot
                co_ps = c_psum.tile([128, BANK], dt.float32, tag="co")
                # front half: stream matmuls + sigmoid + segmented product.
                # Emitted for ALL groups before any transposes so the PE
                # queue never stalls on a transpose waiting for the DVE
                # chain of an earlier group.  Compositing (om/scan/w) for
                # group gi-1 is interleaved after front(gi) so the DVE
                # queue reaches each scan only after its om is ready, and
                # the scans complete early enough for the transposes.
                a_sbs = {}
                w_sbs = {}
                # one contiguous stream DMA + matmul/sigmoid sweep for the
                # whole frame (groups are adjacent in DRAM); per-group work
                # starts at the segmented product
                gcols_all = nbtot * BANK
                w6_sb = w6_pool.tile([6, gcols_all], dt.float16, tag="w6")
                with tc.high_priority():
                    nc.gpsimd.dma_start(out=w6_sb, in_=w6_d[t])
                sp_sb = sp_pool.tile([128, gcols_all], dt.float16, tag="sp")
                lastg = groups[-1]
                tot_used = lastg["off"] + PITCH * lastg["C"] * lastg["n"]
                for b0 in range(0, nbtot, 3):
                    nbk = min(3, nbtot - b0)
                    s_ps = s_psum.tile([128, 3 * BANK], dt.float32, tag="s")
                    for b in range(nbk):
                        nc.tensor.matmul(
                            s_ps[:, b * BANK:(b + 1) * BANK],
                            lhsT=g6_sb,
                            rhs=w6_sb[:, (b0 + b) * BANK:(b0 + b + 1) * BANK],
                            start=True, stop=True)
                    ncols = min(nbk * BANK, tot_used - b0 * BANK)
                    nc.scalar.activation(
                        sp_sb[:, b0 * BANK:b0 * BANK + ncols],
                        s_ps[:, :ncols], AF.Sigmoid)

                def composite(gi):
                    g = groups[gi]
                    C, n_g = g["C"], g["n"]
                    a_sb = a_sbs[gi]
                    if C == 1:
                        # single prim per tile: w = a, no compositing
                        w_sbs[gi] = a_sb
                        return
                    om_sb = om_pool.tile([128, n_g * (C + 1)], dt.float16,
                                         tag="om")
                    r2 = bass.AP(tensor=om_sb.tensor, offset=om_sb.offset,
                                 ap=[om_sb.ap[0], [C + 1, n_g], [1, 1]])
                    nc.gpsimd.memset(r2, 1.0)
                    om_ap = bass.AP(tensor=om_sb.tensor,
                                    offset=om_sb.offset + 1,
                                    ap=[om_sb.ap[0], [C + 1, n_g], [1, C]])
                    nc.gpsimd.tensor_scalar(om_ap, a_sb, -1.0, 1.0,
                                            ALU.mult, ALU.add)
                    tt_sb = tt_pool.tile([128, n_g * (C + 1)], dt.float16,
                                         tag="tt")
                    nc.vector.tensor_tensor_scan(
                        out=tt_sb, data0=om_sb, data1=d1b[gi],
                        initial=ones16[:, 0:1], op0=ALU.mult, op1=ALU.max)
                    w_sb = w_pool.tile([128, n_g * C], dt.float16, tag="w")
                    t0_ap = bass.AP(tensor=tt_sb.tensor, offset=tt_sb.offset,
                                    ap=[tt_sb.ap[0], [C + 1, n_g], [1, C]])
                    t1_ap = bass.AP(tensor=tt_sb.tensor,
                                    offset=tt_sb.offset + 1,
                                    ap=[tt_sb.ap[0], [C + 1, n_g], [1, C]])
                    nc.gpsimd.tensor_tensor(w_sb, t0_ap, t1_ap, ALU.subtract)
                    w_sbs[gi] = w_sb

                gorder = list(range(len(groups)))
                for gi in gorder:
                    g = groups[gi]
                    C, off, n_g = g["C"], g["off"], g["n"]
                    # segmented product over pitch-13 segments
                    a_sb = a_pool.tile([128, C * n_g], dt.float16,
                                       tag=f"a{gi}")
                    sp_ap = bass.AP(tensor=sp_sb.tensor,
                                    offset=sp_sb.offset + off,
                                    ap=[sp_sb.ap[0], [PITCH, C * n_g],
                                        [1, PITCH]])
                    nc.vector.tensor_reduce(a_sb, sp_ap, AX.X, ALU.mult)
                    a_sbs[gi] = a_sb
                # previous frame's framebuffer drain: its color matmuls
                # complete before this frame's stream matmuls do (PE FIFO),
                # so placed here it never blocks this frame's sigmoids
                if prev_fb is not None:
                    tp, co = prev_fb
                    nc.scalar.copy(
                        fb_all[:, tp * NSLOT * 3:(tp + 1) * NSLOT * 3],
                        co[:, :NSLOT * 3])
                for gi in gorder:
                    composite(gi)
                # transposes + color matmuls
                chunk_i = 0
                for gi in gorder:
                    g = groups[gi]
                    C, n_g = g["C"], g["n"]
                    spc, ckoff = g["spc"], g["ckoff"]
                    w_sb = w_sbs[gi]
                    nchunk = (n_g + spc - 1) // spc
                    for j in range(nchunk):
                        ns = min(spc, n_g - j * spc)
                        ccols = ns * C
                        wt_ps = t_psum.tile([128, 1024], dt.float16, tag="wt")
                        nc.tensor.transpose(
                            wt_ps[:ccols, :128],
                            w_sb[:, j * spc * C:j * spc * C + ccols],
                            ident_sb)
                        wt_sb = wt_pool.tile([128, 128], dt.float16, tag="wts")
                        if chunk_i % 2 == 0:
                            nc.scalar.copy(wt_sb[:ccols, :],
                                           wt_ps[:ccols, :128])
                        else:
                            nc.vector.tensor_copy(wt_sb[:ccols, :],
                                                  wt_ps[:ccols, :128])
                        chunk_i += 1
                        s0 = j * spc
                        nc.tensor.matmul(
                            co_ps[:, (g["s0"] + s0) * 3:
                                  (g["s0"] + s0 + ns) * 3],
                            lhsT=wt_sb[0:ccols, :],
                            rhs=ck_all[0:ccols,
                                       cko + ckoff + 3 * s0:
                                       cko + ckoff + 3 * (s0 + ns)],
                            start=True, stop=True)
                prev_fb = (t, co_ps)
            tp, co = prev_fb
            nc.scalar.copy(
                fb_all[:, tp * NSLOT * 3:(tp + 1) * NSLOT * 3],
                co[:, :NSLOT * 3])
            nc.sync.dma_start(out=out_d[:], in_=fb_all)
    nc.finalize()
    return nc


def _get_program(n_frames, groups, nbtot, cktot):
    key = (n_frames, _groups_key(groups), nbtot, cktot)
    if key not in _CACHE:
        _CACHE[key] = _build_nc(n_frames, groups, nbtot, cktot)
    return _CACHE[key]


def _enable_jax_cache():
    try:
        import jax
        if jax.config.jax_compilation_cache_dir is None:
            jax.config.update("jax_compilation_cache_dir", "/tmp/jax_bass_cache")
            jax.config.update("jax_persistent_cache_min_entry_size_bytes", -1)
            jax.config.update("jax_persistent_cache_min_compile_time_secs", 0.5)
    except Exception:
        pass


def _assemble(outs, order):
    """outs: per-core [128, F*NSLOT*3] device outputs -> [T, slot, pix, 3]."""
    devs = []
    for o in outs:
        d = o.reshape(128, -1, NSLOT, 3)          # [pix, F, slot, ch]
        devs.append(np.ascontiguousarray(d.transpose(1, 2, 0, 3)))
    return np.concatenate(devs, axis=0)


def _unpermute(dev, order):
    """dev [T, slot, 128, 3] -> video [T, H, W, 3] (tile un-permute)."""
    T = dev.shape[0]
    video = np.empty((T, NSLOT, TY, TX, 3), np.float32)
    tt = np.arange(T)[:, None]
    video[tt, order] = dev.reshape(T, NSLOT, TY, TX, 3)
    video = video.reshape(T, NTY, NTX, TY, TX, 3)
    video = video.transpose(0, 1, 3, 2, 4, 5).reshape(T, H, W, 3)
    return video


def kernel(trajectory, colors, alpha, z, csg):
    import time
    from concourse.bass_utils import run_bass_kernel_spmd

    _enable_jax_cache()

    in_maps, plan = _host_prep(
        np.asarray(trajectory), np.asarray(colors), np.asarray(alpha),
        np.asarray(z), np.asarray(csg))
    nc = _get_program(F, plan["groups"], plan["nbtot"], plan["cktot"])
    res = None
    for attempt in range(3):
        try:
            res = run_bass_kernel_spmd(nc, in_maps,
                                       core_ids=list(range(N_CORES)))
            break
        except Exception:
            # transient NRT_EXEC_UNIT_UNRECOVERABLE on first exec in a
            # fresh process; a retry has always succeeded
            if attempt == 2:
                raise
            time.sleep(2.0)
    outs = [res.results[c]["out"] for c in range(N_CORES)]
    dev = _assemble(outs, plan["order"])        # [192, slot, pix, 3]
    video = _unpermute(dev, plan["order"])
    return video[None].astype(np.float32)


if __name__ == "__main__":
    import time
    d = np.load("/root/problem/ref_cache.npz")
    t0 = time.time()
    in_maps, plan = _host_prep(d["trajectory"], d["colors"], d["alpha"],
                               d["z"], d["csg"])
    print(f"host prep: {time.time()-t0:.1f}s nbtot={plan['nbtot']}")
    print("groups:", _groups_key(plan["groups"]))
    t0 = time.time()
    nc = _build_nc(2, plan["groups"], plan["nbtot"], plan["cktot"])
    print(f"build 2f: {time.time()-t0:.1f}s")
